# revision 15
# baseline (speedup 1.0000x reference)
"""DNM_Linear Trainium2 kernel — piecewise-linear bucketing → matmul, v2.

Computes, for x:[B,IN] f32, DNM_W:[OUT,M,IN] f32, q:[OUT,M,IN] f32 (constant qs):
    syn  = relu(K*(x[:,None,None,:]*DNM_W - q))      # [B,OUT,M,IN]
    soma = syn.sum(-1).sum(-1)                        # [B,OUT]
    out  = relu(K*(soma - QS))                        # [B,OUT]
with K=0.5, QS=0.1.

Algorithm: for fixed x, f(w) = relu(x*w - qs) is piecewise-linear in w, so
linearly interpolating each weight onto a G-point grid {g_l} is exact except
in the single grid interval containing the kink qs/x:
    sum_m f(W[o,m,i]) ~= sum_l C[o,i,l] * f(g_l)
where C holds interpolation-coefficient sums over m (host-precomputed from W
alone). Basis choice (v2): f(g_l) = relu(x*g_l - qs) = g_l * relu(x - qs/g_l)
for g_l > 0, and f(g_0) == 0 identically for g_0 == 0 (W >= 0 here), so:
  * level 0 vanishes from the device program entirely,
  * per-level scale K^2*g_l folds into the host-side C, and
  * the final affine needs only the scalar constant -K*QS (the final relu
    never clips on this data: reference outputs are all >~260).
Device program per core (tensor-parallel over OUT, 32 outputs/core):
  * DVE:  v_l = relu(xt - qs/g_l), one fp16 4x-mode tensor_scalar per level
          (NLEV = G-1 = 3 instructions of [128 x 512]),
  * PE :  ps[o,b] += cw_block[128k, 32o].T @ v_l[128k, 128b] over
          NLEV*IN/128 = 12 k-blocks,
  * DVE tail: out = max(ps - K*QS, 0), DMA out.
DMA plan: xt split across the sync+vector HWDGE queues for parallel issue;
cw (96 KB fp16) on the scalar queue in two chunks ordered by consumption.
No PE warmup: with only 12 matmuls the HAM clock never reaches full pstate
within the body anyway, and dropping the warmup block removes ~50 PE-queue
instructions (shorter preamble instruction load).

Sharding: tensor-parallel over OUT — core c computes outputs [32c, 32c+32),
host concatenates the 8 [32, 128] partial outputs and transposes.

Measured end-to-end rel err vs the f32 reference: ~9e-3 (gate: 2e-2).

kernel(**inputs) takes FULL inputs and returns the FULL [128,256] f32 output.
"""

import ml_dtypes
import numpy as np

from concourse import bacc, bass, mybir, tile
from concourse.bass_utils import run_bass_kernel_spmd

B, IN, OUT, M = 128, 512, 256, 16
K, QS = 0.5, 0.1
NCORES = 8
OSH = OUT // NCORES        # 32 outputs per core
G = 4                      # PWL grid points (level 0 at w=0 is device-free)
PEXP = 2.0                 # grid spacing exponent
NLEV = G - 1               # device levels
ITILES = IN // 128         # 4
NKB = NLEV * ITILES        # 12 contraction blocks of 128
F16 = mybir.dt.float16
F32 = mybir.dt.float32
F8 = mybir.dt.float8e4
NPF8 = ml_dtypes.float8_e4m3fn

_cache = {}


def _build_program(thresholds):
    nc = bacc.Bacc("TRN2", target_bir_lowering=False)
    xt_d = nc.dram_tensor("xt", [128, ITILES * B], F16, kind="ExternalInput")
    cw_d = nc.dram_tensor("cw", [128, NKB * OSH], F8, kind="ExternalInput")
    out_d = nc.dram_tensor("out", [OSH, B], F32, kind="ExternalOutput")

    sub = mybir.AluOpType.subtract
    amax = mybir.AluOpType.max
    aadd = mybir.AluOpType.add
    H = ITILES * B // 2  # 256: xt half-width for the split input DMA
    NH = NLEV * 2 * OSH  # cw half-chunk columns (6 blocks)

    with tile.TileContext(nc) as tc:
        with (
            tc.tile_pool(name="const", bufs=1) as cpool,
            tc.tile_pool(name="work", bufs=NLEV) as work,
            tc.tile_pool(name="tail", bufs=1) as tail,
            tc.tile_pool(name="psum", bufs=1, space="PSUM") as pp,
        ):
            xt = cpool.tile([128, ITILES * B], F16, name="xt", tag="xt")
            cwt = cpool.tile([128, NKB * OSH], F8, name="cw", tag="cw")

            # Software-pipelined xt pieces sized so each completion
            # semaphore (~1.3us DMA init + transfer + ~0.9us sem prop after
            # issue) lands just before the PE drains the prior piece:
            # piece a = input tiles t0,t1, then t2, then t3, back-to-back
            # on sync (separate rings → overlapping transfer). cw (fp8,
            # 48 KB) lands in one scalar-queue DMA before any matmul.
            nc.sync.dma_start(xt[:, :H], xt_d[:, :H])
            nc.scalar.dma_start(cwt[:, :], cw_d[:, :])
            nc.sync.dma_start(xt[:, H : H + B], xt_d[:, H : H + B])
            nc.sync.dma_start(xt[:, H + B :], xt_d[:, H + B :])

            ps = pp.tile([OSH, B], F32, name="ps", tag="ps")
            v = [
                work.tile([128, ITILES * B], F16, name=f"v{li}", tag=f"v{li}")
                for li in range(NLEV)
            ]
            # DVE: V ops per xt piece (all levels), in piece order.
            for c0, c1 in ((0, H), (H, H + B), (H + B, ITILES * B)):
                for li in range(NLEV):
                    nc.vector.tensor_scalar(
                        v[li][:, c0:c1], xt[:, c0:c1],
                        float(thresholds[li]), 0.0, sub, amax,
                    )
            # PE: matmul issue order follows the piece order; cw block
            # kb-order in DRAM matches.
            order = [(li, t) for li in range(NLEV) for t in (0, 1)]
            order += [(li, 2) for li in range(NLEV)]
            order += [(li, 3) for li in range(NLEV)]
            for kb, (li, t) in enumerate(order):
                nc.tensor.matmul(
                    ps[:, :],
                    cwt[:, kb * OSH : (kb + 1) * OSH],
                    v[li][:, t * B : (t + 1) * B],
                    start=(kb == 0),
                    stop=(kb == NKB - 1),
                )

            # tail: out = max(ps - K*QS, 0); final relu is a no-op on this
            # data but free here.
            fo = tail.tile([OSH, B], F32, name="fo", tag="fo")
            nc.vector.tensor_scalar(fo[:, :], ps[:, :], -K * QS, 0.0, aadd, amax)
            nc.sync.dma_start(out_d[:, :], fo[:, :])

    nc.compile()
    return nc


def _build_C(W64: np.ndarray, grid: np.ndarray) -> np.ndarray:
    """C[o, i, l]: per-(o,i) sums over m of linear-interp coefficients."""
    j = np.clip(np.searchsorted(grid, W64, side="right") - 1, 0, G - 2)
    g0 = grid[j]
    g1 = grid[j + 1]
    a1 = (W64 - g0) / (g1 - g0)
    a0 = 1.0 - a1
    o_idx = np.arange(OUT)[:, None, None]
    i_idx = np.arange(IN)[None, None, :]
    base = (o_idx * IN + i_idx) * G  # [OUT, 1, IN] broadcast over m
    idx0 = (base + j).ravel()
    idx1 = (base + j + 1).ravel()
    n = OUT * IN * G
    C = np.bincount(idx0, weights=a0.ravel(), minlength=n)
    C += np.bincount(idx1, weights=a1.ravel(), minlength=n)
    return C.reshape(OUT, IN, G)


def _in_maps(x, DNM_W, qs, grid):
    x32 = np.asarray(x, np.float32)
    W64 = np.asarray(DNM_W, np.float64)
    C = _build_C(W64, grid)  # [OUT, IN, G] float64

    # xt[p, t*B + b] = x[b, t*128 + p]
    xt = np.ascontiguousarray(
        x32.T.reshape(ITILES, 128, B).transpose(1, 0, 2).reshape(128, ITILES * B)
    ).astype(np.float16)

    # fold K^2 * g_l into the level-l coefficients; drop level 0 (g_0 == 0)
    Cs = (K * K) * (C[:, :, 1:] * grid[None, None, 1:])  # [OUT, IN, NLEV]
    # cw block kb-order in DRAM matches the kernel's matmul issue order
    order = [(li, t) for li in range(NLEV) for t in (0, 1)]
    order += [(li, 2) for li in range(NLEV)]
    order += [(li, 3) for li in range(NLEV)]
    Csr = Cs.reshape(NCORES, OSH, ITILES, 128, NLEV)  # [c, o, t, p, l]
    blocks = [Csr[:, :, t, :, li] for li, t in order]   # each [c, o, p]
    cw = np.stack(blocks, axis=1)                       # [c, kb, o, p]
    cw = np.ascontiguousarray(cw.transpose(0, 3, 1, 2)) # [c, p, kb, o]
    cw = cw.reshape(NCORES, 128, NKB * OSH).astype(NPF8)
    return [{"xt": xt, "cw": cw[c]} for c in range(NCORES)]


def _grid(DNM_W):
    W = np.asarray(DNM_W, np.float64)
    wmin = min(0.0, float(W.min()))
    wmax = float(W.max())
    if wmax <= wmin:
        wmax = wmin + 1.0
    g = wmin + (np.linspace(0.0, 1.0, G) ** PEXP) * (wmax - wmin)
    # round to f32 so device immediates match the C interpolation nodes
    g = g.astype(np.float32).astype(np.float64)
    for i in range(1, G):  # keep strictly increasing after rounding
        if g[i] <= g[i - 1]:
            g[i] = np.nextafter(g[i - 1], np.inf)
    return g


def _host_exact(x, DNM_W, q):
    """Exact reference math on host (fallback for inputs outside this
    problem's setup: non-constant q or negative weights)."""
    x32 = np.asarray(x, np.float32)
    w32 = np.asarray(DNM_W, np.float32)
    q32 = np.broadcast_to(np.asarray(q, np.float32), w32.shape)
    soma = np.zeros((B, OUT), np.float32)
    for o in range(OUT):
        syn = np.maximum(K * (x32[:, None, :] * w32[o] - q32[o]), 0.0)
        soma[:, o] = syn.sum(axis=(1, 2))
    return np.maximum(K * (soma - QS), 0.0).astype(np.float32)


def _run(x, DNM_W, qs, trace=False):
    grid = _grid(DNM_W)
    # thresholds qs/g_l for device levels l = 1..G-1
    thresholds = [float(np.float32(qs / g)) for g in grid[1:]]
    key = (qs, grid.tobytes())
    if key not in _cache:
        _cache[key] = _build_program(thresholds)
    nc = _cache[key]
    res = run_bass_kernel_spmd(nc, _in_maps(x, DNM_W, qs, grid),
                               list(range(NCORES)), trace=trace)
    # per-core out is [OSH, B] = transposed output shard
    out = np.concatenate([res.results[c]["out"] for c in range(NCORES)], axis=0)
    return np.ascontiguousarray(out.T).astype(np.float32), res


def kernel(x, DNM_W, q):
    q = np.asarray(q, np.float32)
    qs = float(q.reshape(-1)[0])
    if not np.all(q == qs) or float(np.asarray(DNM_W).min()) < 0.0:
        return _host_exact(x, DNM_W, q)
    out, _ = _run(x, DNM_W, qs)
    return out


# revision 19
# speedup vs baseline: 1.0968x; 1.0968x over previous
"""DNM_Linear Trainium2 kernel — piecewise-linear bucketing → matmul, v2.

Computes, for x:[B,IN] f32, DNM_W:[OUT,M,IN] f32, q:[OUT,M,IN] f32 (constant qs):
    syn  = relu(K*(x[:,None,None,:]*DNM_W - q))      # [B,OUT,M,IN]
    soma = syn.sum(-1).sum(-1)                        # [B,OUT]
    out  = relu(K*(soma - QS))                        # [B,OUT]
with K=0.5, QS=0.1.

Algorithm: for fixed x, f(w) = relu(x*w - qs) is piecewise-linear in w, so
linearly interpolating each weight onto a G-point grid {g_l} is exact except
in the single grid interval containing the kink qs/x:
    sum_m f(W[o,m,i]) ~= sum_l C[o,i,l] * f(g_l)
where C holds interpolation-coefficient sums over m (host-precomputed from W
alone). Basis choice (v2): f(g_l) = relu(x*g_l - qs) = g_l * relu(x - qs/g_l)
for g_l > 0, and f(g_0) == 0 identically for g_0 == 0 (W >= 0 here), so:
  * level 0 vanishes from the device program entirely,
  * per-level scale K^2*g_l folds into the host-side C, and
  * the final affine needs only the scalar constant -K*QS (the final relu
    never clips on this data: reference outputs are all >~260).
Device program per core (tensor-parallel over OUT, 32 outputs/core):
  * DVE:  v_l = relu(xt - qs/g_l), one fp16 4x-mode tensor_scalar per level
          (NLEV = G-1 = 3 instructions of [128 x 512]),
  * PE :  ps[o,b] += cw_block[128k, 32o].T @ v_l[128k, 128b] over
          NLEV*IN/128 = 12 k-blocks,
  * DVE tail: out = max(ps - K*QS, 0), DMA out.
DMA plan: xt split across the sync+vector HWDGE queues for parallel issue;
cw (96 KB fp16) on the scalar queue in two chunks ordered by consumption.
No PE warmup: with only 12 matmuls the HAM clock never reaches full pstate
within the body anyway, and dropping the warmup block removes ~50 PE-queue
instructions (shorter preamble instruction load).

Sharding: tensor-parallel over OUT — core c computes outputs [32c, 32c+32),
host concatenates the 8 [32, 128] partial outputs and transposes.

Measured end-to-end rel err vs the f32 reference: ~9e-3 (gate: 2e-2).

kernel(**inputs) takes FULL inputs and returns the FULL [128,256] f32 output.
"""

import ml_dtypes
import numpy as np

from concourse import bacc, bass, mybir, tile
from concourse.bass_utils import run_bass_kernel_spmd

B, IN, OUT, M = 128, 512, 256, 16
K, QS = 0.5, 0.1
NCORES = 8
OSH = OUT // NCORES        # 32 outputs per core
G = 4                      # PWL grid points (level 0 at w=0 is device-free)
PEXP = 2.0                 # grid spacing exponent
NLEV = G - 1               # device levels
ITILES = IN // 128         # 4
NKB = NLEV * ITILES        # 12 contraction blocks of 128
F16 = mybir.dt.float16
F32 = mybir.dt.float32
F8 = mybir.dt.float8e4
NPF8 = ml_dtypes.float8_e4m3fn

_cache = {}


def _build_program(thresholds):
    nc = bacc.Bacc("TRN2", target_bir_lowering=False)
    xt_d = nc.dram_tensor("xt", [128, ITILES * B], F16, kind="ExternalInput")
    cw_d = nc.dram_tensor("cw", [128, NKB * OSH], F8, kind="ExternalInput")
    out_d = nc.dram_tensor("out", [OSH, B], F32, kind="ExternalOutput")

    sub = mybir.AluOpType.subtract
    amax = mybir.AluOpType.max
    aadd = mybir.AluOpType.add
    H = ITILES * B // 2  # 256: xt half-width for the split input DMA
    NH = NLEV * 2 * OSH  # cw half-chunk columns (6 blocks)

    with tile.TileContext(nc) as tc:
        with (
            tc.tile_pool(name="const", bufs=1) as cpool,
            tc.tile_pool(name="work", bufs=NLEV) as work,
            tc.tile_pool(name="tail", bufs=1) as tail,
            tc.tile_pool(name="psum", bufs=1, space="PSUM") as pp,
        ):
            xt = cpool.tile([128, ITILES * B], F16, name="xt", tag="xt")
            cwt = cpool.tile([128, NKB * OSH], F8, name="cw", tag="cw")

            # Software-pipelined xt pieces sized so each completion
            # semaphore (~1.3us DMA init + transfer + ~0.9us sem prop after
            # issue) lands just before the PE drains the prior piece:
            # piece a = input tiles t0,t1, then t2, then t3, back-to-back
            # on sync (separate rings → overlapping transfer). cw (fp8,
            # 48 KB) lands in one scalar-queue DMA before any matmul.
            nc.sync.dma_start(xt[:, :H], xt_d[:, :H])
            nc.scalar.dma_start(cwt[:, :], cw_d[:, :])
            nc.gpsimd.dma_start(xt[:, H:], xt_d[:, H:])

            ps = pp.tile([OSH, B], F32, name="ps", tag="ps")
            v = [
                work.tile([128, ITILES * B], F16, name=f"v{li}", tag=f"v{li}")
                for li in range(NLEV)
            ]
            # DVE: V ops per xt piece (all levels), in piece order.
            for c0, c1 in ((0, H), (H, ITILES * B)):
                for li in range(NLEV):
                    nc.vector.tensor_scalar(
                        v[li][:, c0:c1], xt[:, c0:c1],
                        float(thresholds[li]), 0.0, sub, amax,
                    )
            # PE: matmul issue order follows the piece order; cw block
            # kb-order in DRAM matches.
            order = [(li, t) for li in range(NLEV) for t in (0, 1)]
            order += [(li, t) for li in range(NLEV) for t in (2, 3)]
            for kb, (li, t) in enumerate(order):
                nc.tensor.matmul(
                    ps[:, :],
                    cwt[:, kb * OSH : (kb + 1) * OSH],
                    v[li][:, t * B : (t + 1) * B],
                    start=(kb == 0),
                    stop=(kb == NKB - 1),
                )

            # tail: out = max(ps - K*QS, 0); final relu is a no-op on this
            # data but free here.
            fo = tail.tile([OSH, B], F32, name="fo", tag="fo")
            nc.vector.tensor_scalar(fo[:, :], ps[:, :], -K * QS, 0.0, aadd, amax)
            nc.sync.dma_start(out_d[:, :], fo[:, :])

    nc.compile()
    return nc


def _build_C(W64: np.ndarray, grid: np.ndarray) -> np.ndarray:
    """C[o, i, l]: per-(o,i) sums over m of linear-interp coefficients."""
    j = np.clip(np.searchsorted(grid, W64, side="right") - 1, 0, G - 2)
    g0 = grid[j]
    g1 = grid[j + 1]
    a1 = (W64 - g0) / (g1 - g0)
    a0 = 1.0 - a1
    o_idx = np.arange(OUT)[:, None, None]
    i_idx = np.arange(IN)[None, None, :]
    base = (o_idx * IN + i_idx) * G  # [OUT, 1, IN] broadcast over m
    idx0 = (base + j).ravel()
    idx1 = (base + j + 1).ravel()
    n = OUT * IN * G
    C = np.bincount(idx0, weights=a0.ravel(), minlength=n)
    C += np.bincount(idx1, weights=a1.ravel(), minlength=n)
    return C.reshape(OUT, IN, G)


def _in_maps(x, DNM_W, qs, grid):
    x32 = np.asarray(x, np.float32)
    W64 = np.asarray(DNM_W, np.float64)
    C = _build_C(W64, grid)  # [OUT, IN, G] float64

    # xt[p, t*B + b] = x[b, t*128 + p]
    xt = np.ascontiguousarray(
        x32.T.reshape(ITILES, 128, B).transpose(1, 0, 2).reshape(128, ITILES * B)
    ).astype(np.float16)

    # fold K^2 * g_l into the level-l coefficients; drop level 0 (g_0 == 0)
    Cs = (K * K) * (C[:, :, 1:] * grid[None, None, 1:])  # [OUT, IN, NLEV]
    # cw block kb-order in DRAM matches the kernel's matmul issue order
    order = [(li, t) for li in range(NLEV) for t in (0, 1)]
    order += [(li, t) for li in range(NLEV) for t in (2, 3)]
    Csr = Cs.reshape(NCORES, OSH, ITILES, 128, NLEV)  # [c, o, t, p, l]
    blocks = [Csr[:, :, t, :, li] for li, t in order]   # each [c, o, p]
    cw = np.stack(blocks, axis=1)                       # [c, kb, o, p]
    cw = np.ascontiguousarray(cw.transpose(0, 3, 1, 2)) # [c, p, kb, o]
    cw = cw.reshape(NCORES, 128, NKB * OSH).astype(NPF8)
    return [{"xt": xt, "cw": cw[c]} for c in range(NCORES)]


def _grid(DNM_W):
    W = np.asarray(DNM_W, np.float64)
    wmin = min(0.0, float(W.min()))
    wmax = float(W.max())
    if wmax <= wmin:
        wmax = wmin + 1.0
    g = wmin + (np.linspace(0.0, 1.0, G) ** PEXP) * (wmax - wmin)
    # round to f32 so device immediates match the C interpolation nodes
    g = g.astype(np.float32).astype(np.float64)
    for i in range(1, G):  # keep strictly increasing after rounding
        if g[i] <= g[i - 1]:
            g[i] = np.nextafter(g[i - 1], np.inf)
    return g


def _host_exact(x, DNM_W, q):
    """Exact reference math on host (fallback for inputs outside this
    problem's setup: non-constant q or negative weights)."""
    x32 = np.asarray(x, np.float32)
    w32 = np.asarray(DNM_W, np.float32)
    q32 = np.broadcast_to(np.asarray(q, np.float32), w32.shape)
    soma = np.zeros((B, OUT), np.float32)
    for o in range(OUT):
        syn = np.maximum(K * (x32[:, None, :] * w32[o] - q32[o]), 0.0)
        soma[:, o] = syn.sum(axis=(1, 2))
    return np.maximum(K * (soma - QS), 0.0).astype(np.float32)


def _run(x, DNM_W, qs, trace=False):
    grid = _grid(DNM_W)
    # thresholds qs/g_l for device levels l = 1..G-1
    thresholds = [float(np.float32(qs / g)) for g in grid[1:]]
    key = (qs, grid.tobytes())
    if key not in _cache:
        _cache[key] = _build_program(thresholds)
    nc = _cache[key]
    res = run_bass_kernel_spmd(nc, _in_maps(x, DNM_W, qs, grid),
                               list(range(NCORES)), trace=trace)
    # per-core out is [OSH, B] = transposed output shard
    out = np.concatenate([res.results[c]["out"] for c in range(NCORES)], axis=0)
    return np.ascontiguousarray(out.T).astype(np.float32), res


def kernel(x, DNM_W, q):
    q = np.asarray(q, np.float32)
    qs = float(q.reshape(-1)[0])
    if not np.all(q == qs) or float(np.asarray(DNM_W).min()) < 0.0:
        return _host_exact(x, DNM_W, q)
    out, _ = _run(x, DNM_W, qs)
    return out
